# revision 12
# baseline (speedup 1.0000x reference)
"""Multi-head cross-attention kernel for Trainium2, 8-way SPMD. v3.

Problem (nn_CrossAttention): B=2, N=2048, DIM=1024, HEADS=16, d=64.
  q = queries @ Wq.T + bq ; k,v likewise
  out = concat_heads(softmax(q_h k_h^T / sqrt(DIM)) v_h)      -> [B, N, DIM]

Sharding: batch x head-group. Core c handles batch c//4, heads
(c%4)*4 .. (c%4)*4+4 (256 feature columns of Wq/Wk/Wv). Each core
computes its heads' projections + full attention locally; host
concatenates the per-core [256, 2048] outputs (feature-major) back to
[B, N, DIM]. No cross-core communication.

v3 design (v1 452us -> v2 322us):
  * bf16 operands on the PE (host casts); Wq|Wk|Wv packed into one
    [DIM, 768] tensor and all x DMA'd as [128,1024] chunks (2KB
    partition lines) for full DMA throughput.
  * attention per head-PAIR and 512-query chunk: the two heads' score
    matmuls (K=64) run concurrently in different PE row groups
    (tile_position from base_partition 0/64). One ACT exp (N=1024)
    covers both heads. AV accumulation per head in [65,512] psum via
    the ones-column in v_sb (row 64 = softmax denominator).
  * normalization: DVE reciprocal of row 64 -> [1,512] f32r, K=1 PE
    outer-product broadcast, DVE multiply -> bf16 -> DMA. The tail is
    deferred one (pair,qc) iteration AND emitted before the current
    iteration's reciprocals so the S-slot handoff never waits on the
    6.6us reciprocal pair (v2's ~5us boundary stall).
  * HAM management: warmup matmul burst at t=0, +0-accumulate filler
    matmuls inside DMA-paced projection groups and the attention
    j-loop so the PE array never idles long enough to re-throttle.
PSUM budget: S(2 banks x2 bufs) + AV0/AV1 (1 bank x2 bufs each) = 8.
"""

import contextlib

import numpy as np
import ml_dtypes

import concourse.bass as bass
import concourse.mybir as mybir
import concourse.tile as tile
from concourse.bass_utils import run_bass_kernel_spmd

F32 = mybir.dt.float32
F32R = mybir.dt.float32r
BF16 = mybir.dt.bfloat16
AF = mybir.ActivationFunctionType
NPBF16 = ml_dtypes.bfloat16

B, N, DIM, HEADS = 2, 2048, 1024, 16
D = DIM // HEADS          # 64
N_CORES = 8
HPC = HEADS // (N_CORES // B)   # heads per core = 4
FPC = HPC * D                   # feature cols per core = 256
SCALE = DIM ** -0.5
KT = DIM // 128           # contraction tiles = 8
NT = N // 512             # 512-token chunks = 4
JT = N // 128             # key tiles per head = 16
QC = 512                  # query chunk
NQC = N // QC             # 4

N_DUMMY = 2               # attention filler matmuls per j iteration
N_WARM = 24               # warmup matmuls before phase 1


def build_bass(split=True):
    nc = bass.Bass()
    xqT = nc.declare_dram_parameter("xqT", [DIM, N], BF16, isOutput=False)
    xkT = nc.declare_dram_parameter("xkT", [DIM, N], BF16, isOutput=False)
    xvT = nc.declare_dram_parameter("xvT", [DIM, N], BF16, isOutput=False)
    wA = nc.declare_dram_parameter("wA", [DIM, 3 * FPC], BF16, isOutput=False)
    bq = nc.declare_dram_parameter("bq", [2, 128, 1], F32, isOutput=False)
    bk = nc.declare_dram_parameter("bk", [2, 128, 1], F32, isOutput=False)
    bv = nc.declare_dram_parameter("bv", [FPC], F32, isOutput=False)
    outT = nc.declare_dram_parameter("outT", [FPC, N], BF16, isOutput=True)

    with tile.TileContext(nc) as tc:
        with contextlib.ExitStack() as ctx:
            singles = ctx.enter_context(tc.tile_pool(name="singles", bufs=1))
            chunks = ctx.enter_context(tc.tile_pool(name="chunks", bufs=16))
            pts = ctx.enter_context(tc.tile_pool(name="pts", bufs=4))
            recs = ctx.enter_context(tc.tile_pool(name="recs", bufs=4))
            outs = ctx.enter_context(tc.tile_pool(name="outs", bufs=4))
            ps = ctx.enter_context(tc.tile_pool(name="ps", bufs=1, space="PSUM"))

            # --- weights: one [128, 768] tile per k-tile ------------------
            WOFF = {"wq": 0, "wk": FPC, "wv": 2 * FPC}
            w_r = []
            for k in range(KT):
                wr = singles.tile([128, 3 * FPC], BF16, name=f"wr_{k}",
                                  tag=f"wr_{k}")
                nc.sync.dma_start(out=wr, in_=wA[k * 128:(k + 1) * 128, :])
                w_r.append(wr)

            def wslice(name, lo, hi):
                return lambda k: w_r[k][:, WOFF[name] + lo:WOFF[name] + hi]

            bias_t = {}
            for name, dram in (("bq", bq), ("bk", bk)):
                t = singles.tile([128, 2], F32, name=f"bias_{name}",
                                 tag=f"bias_{name}")
                for m in range(2):
                    nc.sync.dma_start(out=t[:, m:m + 1], in_=dram[m])
                bias_t[name] = t
            bv_b = singles.tile([128, FPC], F32, name="bv_b", tag="bv_b")
            bv_ap = bv[:]
            nc.sync.dma_start(
                out=bv_b,
                in_=bass.AP(tensor=bv_ap.tensor, offset=bv_ap.offset,
                            ap=[[0, 128]] + list(bv_ap.ap)))

            ones_f = singles.tile([128, D], F32, name="ones_f", tag="ones_f")
            nc.vector.memset(ones_f, 1.0)
            ones_r = singles.tile([1, D], F32R, name="ones_r", tag="ones_r")
            nc.vector.tensor_copy(ones_r, ones_f[0:1, :])
            # zero operand for +0-accumulate PE filler matmuls
            zero_w = singles.tile([128, 512], BF16, name="zero_w",
                                  tag="zero_w")
            nc.vector.memset(zero_w, 0.0)

            def dummy_into(out_ap, n_free):
                m = out_ap.partition_size()
                nc.tensor.matmul(out_ap, zero_w[:, 0:m],
                                 zero_w[:, 0:n_free],
                                 start=False, stop=False)

            # persistent projection outputs
            qT = [singles.tile([128, N], BF16, name=f"qT_{g}", tag=f"qT_{g}")
                  for g in range(2)]
            kTt = [singles.tile([128, N], BF16, name=f"kT_{g}", tag=f"kT_{g}")
                   for g in range(2)]
            # v with interleaved ones columns: [128 tokens, 16 jtiles, 4*65]
            v_sb = singles.tile([128, JT, HPC * (D + 1)], F32R, name="v_sb",
                                tag="v_sb")
            nc.vector.tensor_copy(
                v_sb.rearrange("p j (h e) -> p j h e", h=HPC)[:, :, :, D:D + 1],
                ones_f.rearrange("p (j h e) -> p j h e", j=JT, h=HPC))

            # --- HAM warmup: PE busy from t~0 -----------------------------
            warm = ps.tile([128, 512], F32, name="warm", tag="S", bufs=2)
            nc.tensor.matmul(warm, zero_w[:, 0:128], zero_w,
                             start=True, stop=False)
            for _ in range(N_WARM - 2):
                dummy_into(warm, 512)
            nc.tensor.matmul(warm, zero_w[:, 0:128], zero_w,
                             start=False, stop=True)

            # --- input DMA stream ([128,1024] chunks, arrival order) ------
            # ck pair0, ck pair1, cv pair0, cq pair0, cv pair1, cq pair1
            ck = [[None] * KT for _ in range(2)]
            cv = [[None] * KT for _ in range(2)]
            cq = [[None] * KT for _ in range(2)]

            def emit_dma(dst, src, p, nm):
                for k in range(KT):
                    ch = chunks.tile([128, 1024], BF16,
                                     name=f"ch_{nm}_{p}_{k}", tag="ch")
                    nc.sync.dma_start(
                        out=ch,
                        in_=src[k * 128:(k + 1) * 128,
                                p * 1024:(p + 1) * 1024])
                    dst[p][k] = ch

            emit_dma(ck, xkT, 0, "k")
            emit_dma(ck, xkT, 1, "k")
            emit_dma(cv, xvT, 0, "v")
            emit_dma(cq, xqT, 0, "q")
            emit_dma(cv, xvT, 1, "v")
            emit_dma(cq, xqT, 1, "q")

            # --- projection emitters --------------------------------------
            def emit_qk_proj(name, srcs, dst, bias, n):
                pj = ps.tile([128, 2, 512], F32, name=f"pj_{name}_{n}",
                             tag="S", bufs=2)
                cs = slice((n % 2) * 512, (n % 2) * 512 + 512)
                for k in range(KT):
                    for m in range(2):
                        nc.tensor.matmul(
                            pj[:, m, :],
                            wslice(name, m * 128, (m + 1) * 128)(k),
                            srcs[n // 2][k][:, cs],
                            start=(k == 0), stop=(k == KT - 1))
                    if k % 2 == 0:
                        dummy_into(pj[:, 0, :], 512)
                for m in range(2):
                    nc.vector.tensor_scalar_add(
                        dst[m][:, n * 512:(n + 1) * 512], pj[:, m, :],
                        bias_t[bias][:, m:m + 1])

            def emit_v_proj(g):
                # token-major: out[tok, feat] for token tiles 4g..4g+3
                for mt in range(4):
                    jt = g * 4 + mt
                    off = (g % 2) * 512 + mt * 128
                    pv = ps.tile([128, FPC], F32, name=f"pv_{jt}", tag="S",
                                 bufs=2)
                    for k in range(KT):
                        nc.tensor.matmul(
                            pv,
                            cv[g // 2][k][:, off:off + 128],
                            wslice("wv", 0, FPC)(k),
                            start=(k == 0), stop=(k == KT - 1))
                        if k % 2 == 0:
                            dummy_into(pv, FPC)
                    nc.vector.tensor_add(
                        v_sb[:, jt, :].rearrange("p (h e) -> p h e",
                                                 h=HPC)[:, :, 0:D],
                        pv.rearrange("p (h d) -> p h d", h=HPC),
                        bv_b.rearrange("p (h d) -> p h d", h=HPC))

            # phase-1 head: K fully, V(g0,g1), Q(n0,n1)
            for n in range(NT):
                emit_qk_proj("wk", ck, kTt, "bk", n)
            emit_v_proj(0)
            emit_v_proj(1)
            emit_qk_proj("wq", cq, qT, "bq", 0)
            emit_qk_proj("wq", cq, qT, "bq", 1)

            # deferred projection groups, keyed by (pair, qc, after_j)
            fillers = {
                (0, 0, 5): lambda: emit_v_proj(2),
                (0, 0, 8): lambda: emit_v_proj(3),
                (0, 0, 12): lambda: emit_qk_proj("wq", cq, qT, "bq", 2),
                (0, 1, 2): lambda: emit_qk_proj("wq", cq, qT, "bq", 3),
            }

            # --- phase 2: attention per (head-pair, query chunk) ----------
            pending_tail = None

            def emit_tail(g, qc, av_s, rec_l):
                for hh in range(2):
                    h = 2 * g + hh
                    bc = ps.tile([D, QC], F32, name=f"bc_{h}_{qc}", tag="S",
                                 bufs=2)
                    nc.tensor.matmul(
                        bc, ones_r, rec_l[hh],
                        start=True, stop=True)
                    o_sb = outs.tile([D, QC], BF16, name=f"o_{h}_{qc}",
                                     tag="o")
                    nc.vector.tensor_mul(o_sb, av_s[hh], bc)
                    nc.sync.dma_start(
                        out=outT[h * D:(h + 1) * D, qc * QC:(qc + 1) * QC],
                        in_=o_sb)

            for g in range(2):
                for qc in range(NQC):
                    av = [ps.tile([D + 1, QC], F32, name=f"av_{g}_{qc}_{hh}",
                                  tag=f"AV{hh}", bufs=2)
                          for hh in range(2)]
                    qs = slice(qc * QC, (qc + 1) * QC)
                    for j in range(JT):
                        S = ps.tile([128, 2, QC], F32, name=f"S_{g}_{qc}_{j}",
                                    tag="S", bufs=2)
                        js = slice(j * 128, (j + 1) * 128)
                        for hh in range(2):
                            r = slice(hh * D, (hh + 1) * D)
                            nc.tensor.matmul(
                                S[:, hh, :], kTt[g][r, js], qT[g][r, qs],
                                start=True, stop=True)
                        pT = pts.tile([128, 2, QC], F32R,
                                      name=f"pT_{g}_{qc}_{j}", tag="pT")
                        nc.scalar.activation(pT, S, AF.Exp, scale=SCALE)
                        for hh in range(2):
                            h = 2 * g + hh
                            e = slice(h * (D + 1), (h + 1) * (D + 1))
                            nc.tensor.matmul(
                                av[hh], v_sb[:, j, e], pT[:, hh, :],
                                start=(j == 0), stop=(j == JT - 1))
                        if 0 < j < JT - 1:
                            for _ in range(N_DUMMY):
                                dummy_into(av[j % 2], QC)
                        f = fillers.pop((g, qc, j), None)
                        if f is not None:
                            f()
                    # deferred tail first: its DVE mul must not queue
                    # behind the reciprocals (S-slot handoff stall)
                    if pending_tail is not None:
                        emit_tail(*pending_tail)
                    rec_l, av_s = [], []
                    for hh in range(2):
                        a_s = outs.tile([D, QC], F32,
                                        name=f"avs_{g}_{qc}_{hh}",
                                        tag=f"avs{hh}")
                        nc.vector.tensor_copy(a_s, av[hh][0:D, :])
                        av_s.append(a_s)
                        rec = recs.tile([1, QC], F32R,
                                        name=f"rec_{g}_{qc}_{hh}",
                                        tag=f"rec{hh}")
                        with nc.allow_low_precision(reason="f32r==f32 bits"):
                            nc.vector.reciprocal(rec, av[hh][D:D + 1, :])
                        rec_l.append(rec)
                    pending_tail = (g, qc, av_s, rec_l)
            emit_tail(*pending_tail)

    if split:
        split_excess_waits(nc)
    return nc


def split_excess_waits(nc, max_waits=1):
    """This walrus codegen accepts one sync wait per instruction; move any
    excess on_wait conditions onto preceding same-engine NoOps."""
    counter = [0]
    for fn in nc.m.functions:
        for blk in fn.blocks:
            new_insts = []
            for inst in blk.instructions:
                si = inst.sync_info
                if si is not None and si.on_wait and len(si.on_wait) > max_waits:
                    waits = list(si.on_wait)
                    excess, keep = waits[:-max_waits], waits[-max_waits:]
                    for w in excess:
                        nop = mybir.InstNoOp(
                            name=f"waitsplit_{counter[0]}", ins=[], outs=[])
                        counter[0] += 1
                        nop.engine = inst.engine
                        nop.sync_info = mybir.SyncInfo(on_wait=[w], on_update=[])
                        new_insts.append(nop)
                    inst.sync_info = mybir.SyncInfo(
                        on_wait=keep, on_update=list(si.on_update or []))
                new_insts.append(inst)
            blk.instructions = new_insts


def make_in_maps(queries, keys, values, Wq, bq, Wk, bk, Wv, bv):
    in_maps = []
    for c in range(N_CORES):
        b = c // (N_CORES // B)
        fs = (c % (N_CORES // B)) * FPC
        fe = fs + FPC
        wA = np.concatenate(
            [Wq[fs:fe, :].T, Wk[fs:fe, :].T, Wv[fs:fe, :].T], axis=1)
        in_maps.append({
            "xqT": np.ascontiguousarray(queries[b].T.astype(NPBF16)),
            "xkT": np.ascontiguousarray(keys[b].T.astype(NPBF16)),
            "xvT": np.ascontiguousarray(values[b].T.astype(NPBF16)),
            "wA": np.ascontiguousarray(wA.astype(NPBF16)),
            "bq": np.ascontiguousarray(bq[fs:fe]).reshape(2, 128, 1),
            "bk": np.ascontiguousarray(bk[fs:fe]).reshape(2, 128, 1),
            "bv": np.ascontiguousarray(bv[fs:fe]),
        })
    return in_maps


_CACHED_NC = None


def kernel(queries, keys, values, Wq, bq, Wk, bk, Wv, bv):
    global _CACHED_NC
    queries = np.asarray(queries, dtype=np.float32)
    keys = np.asarray(keys, dtype=np.float32)
    values = np.asarray(values, dtype=np.float32)
    Wq = np.asarray(Wq, dtype=np.float32)
    Wk = np.asarray(Wk, dtype=np.float32)
    Wv = np.asarray(Wv, dtype=np.float32)
    bq = np.asarray(bq, dtype=np.float32)
    bk = np.asarray(bk, dtype=np.float32)
    bv = np.asarray(bv, dtype=np.float32)

    if _CACHED_NC is None:
        _CACHED_NC = build_bass()
    nc = _CACHED_NC
    in_maps = make_in_maps(queries, keys, values, Wq, bq, Wk, bk, Wv, bv)
    res = run_bass_kernel_spmd(nc, in_maps, list(range(N_CORES))).results

    out = np.empty((B, N, DIM), dtype=np.float32)
    for c in range(N_CORES):
        b = c // (N_CORES // B)
        fs = (c % (N_CORES // B)) * FPC
        out[b, :, fs:fs + FPC] = res[c]["outT"].astype(np.float32).T
    return out


# revision 14
# speedup vs baseline: 1.3248x; 1.3248x over previous
"""Multi-head cross-attention kernel for Trainium2, 8-way SPMD. v3.

Problem (nn_CrossAttention): B=2, N=2048, DIM=1024, HEADS=16, d=64.
  q = queries @ Wq.T + bq ; k,v likewise
  out = concat_heads(softmax(q_h k_h^T / sqrt(DIM)) v_h)      -> [B, N, DIM]

Sharding: batch x head-group. Core c handles batch c//4, heads
(c%4)*4 .. (c%4)*4+4 (256 feature columns of Wq/Wk/Wv). Each core
computes its heads' projections + full attention locally; host
concatenates the per-core [256, 2048] outputs (feature-major) back to
[B, N, DIM]. No cross-core communication.

v3 design (v1 452us -> v2 322us):
  * bf16 operands on the PE (host casts); Wq|Wk|Wv packed into one
    [DIM, 768] tensor and all x DMA'd as [128,1024] chunks (2KB
    partition lines) for full DMA throughput.
  * attention per head-PAIR and 512-query chunk: the two heads' score
    matmuls (K=64) run concurrently in different PE row groups
    (tile_position from base_partition 0/64). One ACT exp (N=1024)
    covers both heads. AV accumulation per head in [65,512] psum via
    the ones-column in v_sb (row 64 = softmax denominator).
  * normalization: DVE reciprocal of row 64 -> [1,512] f32r, K=1 PE
    outer-product broadcast, DVE multiply -> bf16 -> DMA. The tail is
    deferred one (pair,qc) iteration AND emitted before the current
    iteration's reciprocals so the S-slot handoff never waits on the
    6.6us reciprocal pair (v2's ~5us boundary stall).
  * HAM management: warmup matmul burst at t=0, +0-accumulate filler
    matmuls inside DMA-paced projection groups and the attention
    j-loop so the PE array never idles long enough to re-throttle.
PSUM budget: S(2 banks x2 bufs) + AV0/AV1 (1 bank x2 bufs each) = 8.
"""

import contextlib

import numpy as np
import ml_dtypes

import concourse.bass as bass
import concourse.mybir as mybir
import concourse.tile as tile
from concourse.bass_utils import run_bass_kernel_spmd

F32 = mybir.dt.float32
F32R = mybir.dt.float32r
BF16 = mybir.dt.bfloat16
AF = mybir.ActivationFunctionType
NPBF16 = ml_dtypes.bfloat16

B, N, DIM, HEADS = 2, 2048, 1024, 16
D = DIM // HEADS          # 64
N_CORES = 8
HPC = HEADS // (N_CORES // B)   # heads per core = 4
FPC = HPC * D                   # feature cols per core = 256
SCALE = DIM ** -0.5
KT = DIM // 128           # contraction tiles = 8
NT = N // 512             # 512-token chunks = 4
JT = N // 128             # key tiles per head = 16
QC = 512                  # query chunk
NQC = N // QC             # 4

N_DUMMY = 0               # attention filler matmuls per j iteration
N_WARM = 24               # warmup matmuls before phase 1


def build_bass(split=True):
    nc = bass.Bass()
    xqT = nc.declare_dram_parameter("xqT", [DIM, N], BF16, isOutput=False)
    xkT = nc.declare_dram_parameter("xkT", [DIM, N], BF16, isOutput=False)
    xvT = nc.declare_dram_parameter("xvT", [DIM, N], BF16, isOutput=False)
    wA = nc.declare_dram_parameter("wA", [DIM, 3 * FPC], BF16, isOutput=False)
    bq = nc.declare_dram_parameter("bq", [2, 128, 1], F32, isOutput=False)
    bk = nc.declare_dram_parameter("bk", [2, 128, 1], F32, isOutput=False)
    bv = nc.declare_dram_parameter("bv", [FPC], F32, isOutput=False)
    outT = nc.declare_dram_parameter("outT", [FPC, N], BF16, isOutput=True)

    with tile.TileContext(nc) as tc:
        with contextlib.ExitStack() as ctx:
            singles = ctx.enter_context(tc.tile_pool(name="singles", bufs=1))
            chunks = ctx.enter_context(tc.tile_pool(name="chunks", bufs=48))
            pts = ctx.enter_context(tc.tile_pool(name="pts", bufs=4))
            recs = ctx.enter_context(tc.tile_pool(name="recs", bufs=4))
            outs = ctx.enter_context(tc.tile_pool(name="outs", bufs=4))
            ps = ctx.enter_context(tc.tile_pool(name="ps", bufs=1, space="PSUM"))

            # --- weights: one [128, 768] tile per k-tile ------------------
            WOFF = {"wq": 0, "wk": FPC, "wv": 2 * FPC}
            w_r = []
            for k in range(KT):
                wr = singles.tile([128, 3 * FPC], BF16, name=f"wr_{k}",
                                  tag=f"wr_{k}")
                nc.sync.dma_start(out=wr, in_=wA[k * 128:(k + 1) * 128, :])
                w_r.append(wr)

            def wslice(name, lo, hi):
                return lambda k: w_r[k][:, WOFF[name] + lo:WOFF[name] + hi]

            bias_t = {}
            for name, dram in (("bq", bq), ("bk", bk)):
                t = singles.tile([128, 2], F32, name=f"bias_{name}",
                                 tag=f"bias_{name}")
                for m in range(2):
                    nc.gpsimd.dma_start(out=t[:, m:m + 1], in_=dram[m])
                bias_t[name] = t
            bv_b = singles.tile([128, FPC], F32, name="bv_b", tag="bv_b")
            bv_ap = bv[:]
            nc.gpsimd.dma_start(
                out=bv_b,
                in_=bass.AP(tensor=bv_ap.tensor, offset=bv_ap.offset,
                            ap=[[0, 128]] + list(bv_ap.ap)))

            ones_f = singles.tile([128, D], F32, name="ones_f", tag="ones_f")
            nc.vector.memset(ones_f, 1.0)
            ones_r = singles.tile([1, D], F32R, name="ones_r", tag="ones_r")
            nc.vector.tensor_copy(ones_r, ones_f[0:1, :])
            # zero operand for +0-accumulate PE filler matmuls
            zero_w = singles.tile([128, 512], BF16, name="zero_w",
                                  tag="zero_w")
            nc.vector.memset(zero_w, 0.0)

            def dummy_into(out_ap, n_free):
                m = out_ap.partition_size()
                nc.tensor.matmul(out_ap, zero_w[:, 0:m],
                                 zero_w[:, 0:n_free],
                                 start=False, stop=False)

            # persistent projection outputs
            qT = [singles.tile([128, N], BF16, name=f"qT_{g}", tag=f"qT_{g}")
                  for g in range(2)]
            kTt = [singles.tile([128, N], BF16, name=f"kT_{g}", tag=f"kT_{g}")
                   for g in range(2)]
            # v with interleaved ones columns: [128 tokens, 16 jtiles, 4*65]
            v_sb = singles.tile([128, JT, HPC * (D + 1)], F32R, name="v_sb",
                                tag="v_sb")
            nc.vector.tensor_copy(
                v_sb.rearrange("p j (h e) -> p j h e", h=HPC)[:, :, :, D:D + 1],
                ones_f.rearrange("p (j h e) -> p j h e", j=JT, h=HPC))

            # --- HAM warmup: PE busy from t~0 -----------------------------
            warm = ps.tile([128, 512], F32, name="warm", tag="S", bufs=2)
            nc.tensor.matmul(warm, zero_w[:, 0:128], zero_w,
                             start=True, stop=False)
            for _ in range(N_WARM - 2):
                dummy_into(warm, 512)
            nc.tensor.matmul(warm, zero_w[:, 0:128], zero_w,
                             start=False, stop=True)

            # --- input DMA stream ([128,1024] chunks, arrival order) ------
            # ck pair0, ck pair1, cv pair0, cq pair0, cv pair1, cq pair1
            ck = [[None] * KT for _ in range(2)]
            cv = [[None] * KT for _ in range(2)]
            cq = [[None] * KT for _ in range(2)]

            def emit_dma(dst, src, p, nm, eng=None):
                eng = eng or nc.sync
                for k in range(KT):
                    ch = chunks.tile([128, 1024], BF16,
                                     name=f"ch_{nm}_{p}_{k}", tag="ch")
                    eng.dma_start(
                        out=ch,
                        in_=src[k * 128:(k + 1) * 128,
                                p * 1024:(p + 1) * 1024])
                    dst[p][k] = ch

            # sync queue: w, xk pair0, xq pair0, xq pair1
            # gpsimd queue: biases, xv pair0, xk pair1, xv pair1
            emit_dma(ck, xkT, 0, "k")
            emit_dma(cv, xvT, 0, "v", eng=nc.gpsimd)
            emit_dma(cq, xqT, 0, "q")
            emit_dma(ck, xkT, 1, "k", eng=nc.gpsimd)
            emit_dma(cq, xqT, 1, "q")
            emit_dma(cv, xvT, 1, "v", eng=nc.gpsimd)

            # --- projection emitters --------------------------------------
            def emit_qk_proj(name, srcs, dst, bias, n):
                pj = ps.tile([128, 2, 512], F32, name=f"pj_{name}_{n}",
                             tag="S", bufs=2)
                cs = slice((n % 2) * 512, (n % 2) * 512 + 512)
                for k in range(KT):
                    for m in range(2):
                        nc.tensor.matmul(
                            pj[:, m, :],
                            wslice(name, m * 128, (m + 1) * 128)(k),
                            srcs[n // 2][k][:, cs],
                            start=(k == 0), stop=(k == KT - 1))
                for m in range(2):
                    nc.vector.tensor_scalar_add(
                        dst[m][:, n * 512:(n + 1) * 512], pj[:, m, :],
                        bias_t[bias][:, m:m + 1])

            def emit_v_proj_jt(jt):
                # token-major: out[tok, feat] for token tile jt
                off = (jt % 8) * 128
                pv = ps.tile([128, FPC], F32, name=f"pv_{jt}", tag="S",
                             bufs=2)
                for k in range(KT):
                    nc.tensor.matmul(
                        pv,
                        cv[jt // 8][k][:, off:off + 128],
                        wslice("wv", 0, FPC)(k),
                        start=(k == 0), stop=(k == KT - 1))
                nc.vector.tensor_add(
                    v_sb[:, jt, :].rearrange("p (h e) -> p h e",
                                             h=HPC)[:, :, 0:D],
                    pv.rearrange("p (h d) -> p h d", h=HPC),
                    bv_b.rearrange("p (h d) -> p h d", h=HPC))

            # phase-1 head: K n0/n1, V jt0-7, Q n0; rest deferred
            emit_qk_proj("wk", ck, kTt, "bk", 0)
            emit_qk_proj("wk", ck, kTt, "bk", 1)
            for jt in range(8):
                emit_v_proj_jt(jt)
            emit_qk_proj("wq", cq, qT, "bq", 0)

            # deferred projection groups, keyed by (pair, qc, after_j)
            fillers = {
                (0, 0, 2): lambda: emit_qk_proj("wk", ck, kTt, "bk", 2),
                (0, 0, 4): lambda: emit_qk_proj("wk", ck, kTt, "bk", 3),
                (0, 0, 14): lambda: emit_qk_proj("wq", cq, qT, "bq", 1),
                (0, 1, 1): lambda: emit_qk_proj("wq", cq, qT, "bq", 2),
                (0, 2, 1): lambda: emit_qk_proj("wq", cq, qT, "bq", 3),
            }
            for i, jt in enumerate(range(8, 16)):
                fillers[(0, 0, 6 + i)] = (
                    lambda jt=jt: emit_v_proj_jt(jt))

            # --- phase 2: attention per (head-pair, query chunk) ----------
            pending_tail = None

            def emit_tail(g, qc, av_s, rec_l):
                for hh in range(2):
                    h = 2 * g + hh
                    bc = ps.tile([D, QC], F32, name=f"bc_{h}_{qc}", tag="S",
                                 bufs=2)
                    nc.tensor.matmul(
                        bc, ones_r, rec_l[hh],
                        start=True, stop=True)
                    o_sb = outs.tile([D, QC], BF16, name=f"o_{h}_{qc}",
                                     tag="o")
                    nc.vector.tensor_mul(o_sb, av_s[hh], bc)
                    nc.sync.dma_start(
                        out=outT[h * D:(h + 1) * D, qc * QC:(qc + 1) * QC],
                        in_=o_sb)

            for g in range(2):
                for qc in range(NQC):
                    av = [ps.tile([D + 1, QC], F32, name=f"av_{g}_{qc}_{hh}",
                                  tag=f"AV{hh}", bufs=2)
                          for hh in range(2)]
                    qs = slice(qc * QC, (qc + 1) * QC)
                    for j in range(JT):
                        S = ps.tile([128, 2, QC], F32, name=f"S_{g}_{qc}_{j}",
                                    tag="S", bufs=2)
                        js = slice(j * 128, (j + 1) * 128)
                        for hh in range(2):
                            r = slice(hh * D, (hh + 1) * D)
                            nc.tensor.matmul(
                                S[:, hh, :], kTt[g][r, js], qT[g][r, qs],
                                start=True, stop=True)
                        pT = pts.tile([128, 2, QC], F32R,
                                      name=f"pT_{g}_{qc}_{j}", tag="pT")
                        nc.scalar.activation(pT, S, AF.Exp, scale=SCALE)
                        for hh in range(2):
                            h = 2 * g + hh
                            e = slice(h * (D + 1), (h + 1) * (D + 1))
                            nc.tensor.matmul(
                                av[hh], v_sb[:, j, e], pT[:, hh, :],
                                start=(j == 0), stop=(j == JT - 1))
                        if 0 < j < JT - 1:
                            for _ in range(N_DUMMY):
                                dummy_into(av[j % 2], QC)
                        f = fillers.pop((g, qc, j), None)
                        if f is not None:
                            f()
                    # deferred tail first: its DVE mul must not queue
                    # behind the reciprocals (S-slot handoff stall)
                    if pending_tail is not None:
                        emit_tail(*pending_tail)
                    rec_l, av_s = [], []
                    for hh in range(2):
                        a_s = outs.tile([D, QC], F32,
                                        name=f"avs_{g}_{qc}_{hh}",
                                        tag=f"avs{hh}")
                        nc.vector.tensor_copy(a_s, av[hh][0:D, :])
                        av_s.append(a_s)
                        rec = recs.tile([1, QC], F32R,
                                        name=f"rec_{g}_{qc}_{hh}",
                                        tag=f"rec{hh}")
                        with nc.allow_low_precision(reason="f32r==f32 bits"):
                            nc.vector.reciprocal(rec, av[hh][D:D + 1, :])
                        rec_l.append(rec)
                    pending_tail = (g, qc, av_s, rec_l)
            emit_tail(*pending_tail)

    if split:
        split_excess_waits(nc)
    return nc


def split_excess_waits(nc, max_waits=1):
    """This walrus codegen accepts one sync wait per instruction; move any
    excess on_wait conditions onto preceding same-engine NoOps."""
    counter = [0]
    for fn in nc.m.functions:
        for blk in fn.blocks:
            new_insts = []
            for inst in blk.instructions:
                si = inst.sync_info
                if si is not None and si.on_wait and len(si.on_wait) > max_waits:
                    waits = list(si.on_wait)
                    excess, keep = waits[:-max_waits], waits[-max_waits:]
                    for w in excess:
                        nop = mybir.InstNoOp(
                            name=f"waitsplit_{counter[0]}", ins=[], outs=[])
                        counter[0] += 1
                        nop.engine = inst.engine
                        nop.sync_info = mybir.SyncInfo(on_wait=[w], on_update=[])
                        new_insts.append(nop)
                    inst.sync_info = mybir.SyncInfo(
                        on_wait=keep, on_update=list(si.on_update or []))
                new_insts.append(inst)
            blk.instructions = new_insts


def make_in_maps(queries, keys, values, Wq, bq, Wk, bk, Wv, bv):
    in_maps = []
    for c in range(N_CORES):
        b = c // (N_CORES // B)
        fs = (c % (N_CORES // B)) * FPC
        fe = fs + FPC
        wA = np.concatenate(
            [Wq[fs:fe, :].T, Wk[fs:fe, :].T, Wv[fs:fe, :].T], axis=1)
        in_maps.append({
            "xqT": np.ascontiguousarray(queries[b].T.astype(NPBF16)),
            "xkT": np.ascontiguousarray(keys[b].T.astype(NPBF16)),
            "xvT": np.ascontiguousarray(values[b].T.astype(NPBF16)),
            "wA": np.ascontiguousarray(wA.astype(NPBF16)),
            "bq": np.ascontiguousarray(bq[fs:fe]).reshape(2, 128, 1),
            "bk": np.ascontiguousarray(bk[fs:fe]).reshape(2, 128, 1),
            "bv": np.ascontiguousarray(bv[fs:fe]),
        })
    return in_maps


_CACHED_NC = None


def kernel(queries, keys, values, Wq, bq, Wk, bk, Wv, bv):
    global _CACHED_NC
    queries = np.asarray(queries, dtype=np.float32)
    keys = np.asarray(keys, dtype=np.float32)
    values = np.asarray(values, dtype=np.float32)
    Wq = np.asarray(Wq, dtype=np.float32)
    Wk = np.asarray(Wk, dtype=np.float32)
    Wv = np.asarray(Wv, dtype=np.float32)
    bq = np.asarray(bq, dtype=np.float32)
    bk = np.asarray(bk, dtype=np.float32)
    bv = np.asarray(bv, dtype=np.float32)

    if _CACHED_NC is None:
        _CACHED_NC = build_bass()
    nc = _CACHED_NC
    in_maps = make_in_maps(queries, keys, values, Wq, bq, Wk, bk, Wv, bv)
    res = run_bass_kernel_spmd(nc, in_maps, list(range(N_CORES))).results

    out = np.empty((B, N, DIM), dtype=np.float32)
    for c in range(N_CORES):
        b = c // (N_CORES // B)
        fs = (c % (N_CORES // B)) * FPC
        out[b, :, fs:fs + FPC] = res[c]["outT"].astype(np.float32).T
    return out


# revision 21
# speedup vs baseline: 1.3607x; 1.0271x over previous
"""Multi-head cross-attention kernel for Trainium2, 8-way SPMD. v3.

Problem (nn_CrossAttention): B=2, N=2048, DIM=1024, HEADS=16, d=64.
  q = queries @ Wq.T + bq ; k,v likewise
  out = concat_heads(softmax(q_h k_h^T / sqrt(DIM)) v_h)      -> [B, N, DIM]

Sharding: batch x head-group. Core c handles batch c//4, heads
(c%4)*4 .. (c%4)*4+4 (256 feature columns of Wq/Wk/Wv). Each core
computes its heads' projections + full attention locally; host
concatenates the per-core [256, 2048] outputs (feature-major) back to
[B, N, DIM]. No cross-core communication.

v3 design (v1 452us -> v2 322us):
  * bf16 operands on the PE (host casts); Wq|Wk|Wv packed into one
    [DIM, 768] tensor and all x DMA'd as [128,1024] chunks (2KB
    partition lines) for full DMA throughput.
  * attention per head-PAIR and 512-query chunk: the two heads' score
    matmuls (K=64) run concurrently in different PE row groups
    (tile_position from base_partition 0/64). One ACT exp (N=1024)
    covers both heads. AV accumulation per head in [65,512] psum via
    the ones-column in v_sb (row 64 = softmax denominator).
  * normalization: DVE reciprocal of row 64 -> [1,512] f32r, K=1 PE
    outer-product broadcast, DVE multiply -> bf16 -> DMA. The tail is
    deferred one (pair,qc) iteration AND emitted before the current
    iteration's reciprocals so the S-slot handoff never waits on the
    6.6us reciprocal pair (v2's ~5us boundary stall).
  * HAM management: warmup matmul burst at t=0, +0-accumulate filler
    matmuls inside DMA-paced projection groups and the attention
    j-loop so the PE array never idles long enough to re-throttle.
PSUM budget: S(2 banks x2 bufs) + AV0/AV1 (1 bank x2 bufs each) = 8.
"""

import contextlib

import numpy as np
import ml_dtypes

import concourse.bass as bass
import concourse.mybir as mybir
import concourse.tile as tile
from concourse.bass_utils import run_bass_kernel_spmd

F32 = mybir.dt.float32
F32R = mybir.dt.float32r
BF16 = mybir.dt.bfloat16
AF = mybir.ActivationFunctionType
NPBF16 = ml_dtypes.bfloat16

B, N, DIM, HEADS = 2, 2048, 1024, 16
D = DIM // HEADS          # 64
N_CORES = 8
HPC = HEADS // (N_CORES // B)   # heads per core = 4
FPC = HPC * D                   # feature cols per core = 256
SCALE = DIM ** -0.5
KT = DIM // 128           # contraction tiles = 8
NT = N // 512             # 512-token chunks = 4
JT = N // 128             # key tiles per head = 16
QC = 512                  # query chunk
NQC = N // QC             # 4

N_DUMMY = 0               # attention filler matmuls per j iteration
N_WARM = 12               # warmup matmuls before phase 1


def build_bass(split=True):
    nc = bass.Bass()
    xqT = nc.declare_dram_parameter("xqT", [DIM, N], BF16, isOutput=False)
    xkT = nc.declare_dram_parameter("xkT", [DIM, N], BF16, isOutput=False)
    xvT = nc.declare_dram_parameter("xvT", [DIM, N], BF16, isOutput=False)
    wA = nc.declare_dram_parameter("wA", [DIM, 3 * FPC], BF16, isOutput=False)
    bq = nc.declare_dram_parameter("bq", [2, 128, 1], F32, isOutput=False)
    bk = nc.declare_dram_parameter("bk", [2, 128, 1], F32, isOutput=False)
    bv = nc.declare_dram_parameter("bv", [FPC], F32, isOutput=False)
    outT = nc.declare_dram_parameter("outT", [FPC, N], BF16, isOutput=True)

    with tile.TileContext(nc) as tc:
        with contextlib.ExitStack() as ctx:
            singles = ctx.enter_context(tc.tile_pool(name="singles", bufs=1))
            chunks = ctx.enter_context(tc.tile_pool(name="chunks", bufs=36))
            pts = ctx.enter_context(tc.tile_pool(name="pts", bufs=4))
            recs = ctx.enter_context(tc.tile_pool(name="recs", bufs=2))
            outs = ctx.enter_context(tc.tile_pool(name="outs", bufs=4))
            ps = ctx.enter_context(tc.tile_pool(name="ps", bufs=1, space="PSUM"))

            # --- weights: one [128, 768] tile per k-tile ------------------
            WOFF = {"wq": 0, "wk": FPC, "wv": 2 * FPC}
            w_r = []
            for k in range(KT):
                wr = singles.tile([128, 3 * FPC], BF16, name=f"wr_{k}",
                                  tag=f"wr_{k}")
                nc.scalar.dma_start(out=wr, in_=wA[k * 128:(k + 1) * 128, :])
                w_r.append(wr)

            def wslice(name, lo, hi):
                return lambda k: w_r[k][:, WOFF[name] + lo:WOFF[name] + hi]

            bias_t = {}
            for name, dram in (("bq", bq), ("bk", bk)):
                t = singles.tile([128, 2], F32, name=f"bias_{name}",
                                 tag=f"bias_{name}")
                for m in range(2):
                    nc.gpsimd.dma_start(out=t[:, m:m + 1], in_=dram[m])
                bias_t[name] = t
            bv_b = singles.tile([128, FPC], F32, name="bv_b", tag="bv_b")
            bv_ap = bv[:]
            nc.gpsimd.dma_start(
                out=bv_b,
                in_=bass.AP(tensor=bv_ap.tensor, offset=bv_ap.offset,
                            ap=[[0, 128]] + list(bv_ap.ap)))

            ones_f = singles.tile([128, D], F32, name="ones_f", tag="ones_f")
            nc.vector.memset(ones_f, 1.0)
            ones_r = singles.tile([1, D], F32R, name="ones_r", tag="ones_r")
            nc.vector.tensor_copy(ones_r, ones_f[0:1, :])
            # zero operand for +0-accumulate PE filler matmuls
            zero_w = singles.tile([128, 512], BF16, name="zero_w",
                                  tag="zero_w")
            nc.vector.memset(zero_w, 0.0)

            def dummy_into(out_ap, n_free):
                m = out_ap.partition_size()
                nc.tensor.matmul(out_ap, zero_w[:, 0:m],
                                 zero_w[:, 0:n_free],
                                 start=False, stop=False)

            # persistent projection outputs
            qT = [singles.tile([128, N], BF16, name=f"qT_{g}", tag=f"qT_{g}")
                  for g in range(2)]
            kTt = [singles.tile([128, N], BF16, name=f"kT_{g}", tag=f"kT_{g}")
                   for g in range(2)]
            # v with interleaved ones columns: [128 tokens, 16 jtiles, 4*65]
            v_sb = singles.tile([128, JT, HPC * (D + 1)], F32R, name="v_sb",
                                tag="v_sb")
            nc.vector.tensor_copy(
                v_sb.rearrange("p j (h e) -> p j h e", h=HPC)[:, :, :, D:D + 1],
                ones_f.rearrange("p (j h e) -> p j h e", j=JT, h=HPC))

            # --- HAM warmup: PE busy from t~0 -----------------------------
            warm = ps.tile([128, 512], F32, name="warm", tag="S", bufs=2)
            nc.tensor.matmul(warm, zero_w[:, 0:128], zero_w,
                             start=True, stop=False)
            for _ in range(N_WARM - 2):
                dummy_into(warm, 512)
            nc.tensor.matmul(warm, zero_w[:, 0:128], zero_w,
                             start=False, stop=True)

            # --- input DMA stream ([128,1024] chunks, arrival order) ------
            # ck pair0, ck pair1, cv pair0, cq pair0, cv pair1, cq pair1
            ck = [[None] * KT for _ in range(2)]
            cv = [[None] * KT for _ in range(2)]
            cq = [[None] * KT for _ in range(2)]

            def emit_dma(dst, src, p, nm, eng=None):
                eng = eng or nc.sync
                for k in range(KT):
                    ch = chunks.tile([128, 1024], BF16,
                                     name=f"ch_{nm}_{p}_{k}", tag="ch")
                    eng.dma_start(
                        out=ch,
                        in_=src[k * 128:(k + 1) * 128,
                                p * 1024:(p + 1) * 1024])
                    dst[p][k] = ch

            # sync queue: w, xk pair0, xq pair0, xq pair1
            # gpsimd queue: biases, xv pair0, xk pair1, xv pair1
            emit_dma(ck, xkT, 0, "k", eng=nc.scalar)
            emit_dma(cv, xvT, 0, "v", eng=nc.gpsimd)
            emit_dma(cq, xqT, 0, "q", eng=nc.scalar)
            emit_dma(ck, xkT, 1, "k", eng=nc.gpsimd)
            emit_dma(cq, xqT, 1, "q", eng=nc.scalar)
            emit_dma(cv, xvT, 1, "v", eng=nc.gpsimd)

            # --- projection emitters --------------------------------------
            def emit_qk_proj(name, srcs, dst, bias, n):
                pj = ps.tile([128, 2, 512], F32, name=f"pj_{name}_{n}",
                             tag="S", bufs=2)
                cs = slice((n % 2) * 512, (n % 2) * 512 + 512)
                for k in range(KT):
                    for m in range(2):
                        nc.tensor.matmul(
                            pj[:, m, :],
                            wslice(name, m * 128, (m + 1) * 128)(k),
                            srcs[n // 2][k][:, cs],
                            start=(k == 0), stop=(k == KT - 1))
                for m in range(2):
                    nc.vector.tensor_scalar_add(
                        dst[m][:, n * 512:(n + 1) * 512], pj[:, m, :],
                        bias_t[bias][:, m:m + 1])

            def emit_v_proj_jt(jt):
                # token-major: out[tok, feat] for token tile jt
                off = (jt % 8) * 128
                pv = ps.tile([128, FPC], F32, name=f"pv_{jt}", tag="S",
                             bufs=2)
                for k in range(KT):
                    nc.tensor.matmul(
                        pv,
                        cv[jt // 8][k][:, off:off + 128],
                        wslice("wv", 0, FPC)(k),
                        start=(k == 0), stop=(k == KT - 1))
                nc.vector.tensor_add(
                    v_sb[:, jt, :].rearrange("p (h e) -> p h e",
                                             h=HPC)[:, :, 0:D],
                    pv.rearrange("p (h d) -> p h d", h=HPC),
                    bv_b.rearrange("p (h d) -> p h d", h=HPC))

            # phase-1 head: K fully (DMA-shadowed), V jt0-9, Q n0
            for n in range(NT):
                emit_qk_proj("wk", ck, kTt, "bk", n)
            for jt in range(10):
                emit_v_proj_jt(jt)
            emit_qk_proj("wq", cq, qT, "bq", 0)

            # deferred projection groups, keyed by (pair, qc, after_j)
            fillers = {
                (0, 0, 13): lambda: emit_qk_proj("wq", cq, qT, "bq", 1),
                (0, 1, 1): lambda: emit_qk_proj("wq", cq, qT, "bq", 2),
                (0, 2, 1): lambda: emit_qk_proj("wq", cq, qT, "bq", 3),
            }
            for i, jt in enumerate(range(10, 16)):
                fillers[(0, 0, 6 + i)] = (
                    lambda jt=jt: emit_v_proj_jt(jt))

            # --- phase 2: attention per (head-pair, query chunk) ----------
            pending_tail = None

            def emit_tail(g, qc, av_s, rec_l):
                for hh in range(2):
                    h = 2 * g + hh
                    bc = ps.tile([D, QC], F32, name=f"bc_{h}_{qc}", tag="S",
                                 bufs=2)
                    nc.tensor.matmul(
                        bc, ones_r, rec_l[hh],
                        start=True, stop=True)
                    o_sb = outs.tile([D, QC], BF16, name=f"o_{h}_{qc}",
                                     tag="o")
                    nc.vector.tensor_mul(o_sb, av_s[hh], bc)
                    nc.sync.dma_start(
                        out=outT[h * D:(h + 1) * D, qc * QC:(qc + 1) * QC],
                        in_=o_sb)

            for g in range(2):
                for qc in range(NQC):
                    av = [ps.tile([D + 1, QC], F32, name=f"av_{g}_{qc}_{hh}",
                                  tag=f"AV{hh}", bufs=2)
                          for hh in range(2)]
                    qs = slice(qc * QC, (qc + 1) * QC)
                    for j in range(JT):
                        S = ps.tile([128, 2, QC], F32, name=f"S_{g}_{qc}_{j}",
                                    tag="S", bufs=2)
                        js = slice(j * 128, (j + 1) * 128)
                        for hh in range(2):
                            r = slice(hh * D, (hh + 1) * D)
                            nc.tensor.matmul(
                                S[:, hh, :], kTt[g][r, js], qT[g][r, qs],
                                start=True, stop=True)
                        pT = pts.tile([128, 2, QC], F32R,
                                      name=f"pT_{g}_{qc}_{j}", tag="pT")
                        nc.scalar.activation(pT, S, AF.Exp, scale=SCALE)
                        for hh in range(2):
                            h = 2 * g + hh
                            e = slice(h * (D + 1), (h + 1) * (D + 1))
                            nc.tensor.matmul(
                                av[hh], v_sb[:, j, e], pT[:, hh, :],
                                start=(j == 0), stop=(j == JT - 1))
                        if 0 < j < JT - 1:
                            for _ in range(N_DUMMY):
                                dummy_into(av[j % 2], QC)
                        f = fillers.pop((g, qc, j), None)
                        if f is not None:
                            f()
                    # deferred tail first: its DVE mul must not queue
                    # behind the reciprocals (S-slot handoff stall)
                    if pending_tail is not None:
                        emit_tail(*pending_tail)
                    rec_l, av_s = [], []
                    for hh in range(2):
                        a_s = outs.tile([D, QC], F32,
                                        name=f"avs_{g}_{qc}_{hh}",
                                        tag=f"avs{hh}", bufs=2)
                        nc.vector.tensor_copy(a_s, av[hh][0:D, :])
                        av_s.append(a_s)
                        rec = recs.tile([1, QC], F32R,
                                        name=f"rec_{g}_{qc}_{hh}",
                                        tag=f"rec{hh}")
                        if (g, qc) == (1, NQC - 1):
                            # final chunk: ACT is idle; 1/d = exp(-ln d)
                            # (natural_log_exp set holds both fns)
                            lt = recs.tile([1, QC], F32,
                                           name=f"lt_{hh}", tag=f"lt{hh}")
                            nc.scalar.activation(
                                lt, av[hh][D:D + 1, :], AF.Ln)
                            nc.scalar.activation(rec, lt, AF.Exp, scale=-1.0)
                        else:
                            with nc.allow_low_precision(reason="f32r=f32"):
                                nc.vector.reciprocal(rec, av[hh][D:D + 1, :])
                        rec_l.append(rec)
                    pending_tail = (g, qc, av_s, rec_l)
            emit_tail(*pending_tail)

    if split:
        split_excess_waits(nc)
    return nc


def split_excess_waits(nc, max_waits=1):
    """This walrus codegen accepts one sync wait per instruction; move any
    excess on_wait conditions onto preceding same-engine NoOps."""
    counter = [0]
    for fn in nc.m.functions:
        for blk in fn.blocks:
            new_insts = []
            for inst in blk.instructions:
                si = inst.sync_info
                if si is not None and si.on_wait and len(si.on_wait) > max_waits:
                    waits = list(si.on_wait)
                    excess, keep = waits[:-max_waits], waits[-max_waits:]
                    for w in excess:
                        nop = mybir.InstNoOp(
                            name=f"waitsplit_{counter[0]}", ins=[], outs=[])
                        counter[0] += 1
                        nop.engine = inst.engine
                        nop.sync_info = mybir.SyncInfo(on_wait=[w], on_update=[])
                        new_insts.append(nop)
                    inst.sync_info = mybir.SyncInfo(
                        on_wait=keep, on_update=list(si.on_update or []))
                new_insts.append(inst)
            blk.instructions = new_insts


def make_in_maps(queries, keys, values, Wq, bq, Wk, bk, Wv, bv):
    in_maps = []
    for c in range(N_CORES):
        b = c // (N_CORES // B)
        fs = (c % (N_CORES // B)) * FPC
        fe = fs + FPC
        wA = np.concatenate(
            [Wq[fs:fe, :].T, Wk[fs:fe, :].T, Wv[fs:fe, :].T], axis=1)
        in_maps.append({
            "xqT": np.ascontiguousarray(queries[b].T.astype(NPBF16)),
            "xkT": np.ascontiguousarray(keys[b].T.astype(NPBF16)),
            "xvT": np.ascontiguousarray(values[b].T.astype(NPBF16)),
            "wA": np.ascontiguousarray(wA.astype(NPBF16)),
            "bq": np.ascontiguousarray(bq[fs:fe]).reshape(2, 128, 1),
            "bk": np.ascontiguousarray(bk[fs:fe]).reshape(2, 128, 1),
            "bv": np.ascontiguousarray(bv[fs:fe]),
        })
    return in_maps


_CACHED_NC = None


def kernel(queries, keys, values, Wq, bq, Wk, bk, Wv, bv):
    global _CACHED_NC
    queries = np.asarray(queries, dtype=np.float32)
    keys = np.asarray(keys, dtype=np.float32)
    values = np.asarray(values, dtype=np.float32)
    Wq = np.asarray(Wq, dtype=np.float32)
    Wk = np.asarray(Wk, dtype=np.float32)
    Wv = np.asarray(Wv, dtype=np.float32)
    bq = np.asarray(bq, dtype=np.float32)
    bk = np.asarray(bk, dtype=np.float32)
    bv = np.asarray(bv, dtype=np.float32)

    if _CACHED_NC is None:
        _CACHED_NC = build_bass()
    nc = _CACHED_NC
    in_maps = make_in_maps(queries, keys, values, Wq, bq, Wk, bk, Wv, bv)
    res = run_bass_kernel_spmd(nc, in_maps, list(range(N_CORES))).results

    out = np.empty((B, N, DIM), dtype=np.float32)
    for c in range(N_CORES):
        b = c // (N_CORES // B)
        fs = (c % (N_CORES // B)) * FPC
        out[b, :, fs:fs + FPC] = res[c]["outT"].astype(np.float32).T
    return out
